# revision 1
# baseline (speedup 1.0000x reference)
"""Trainium2 Bass kernel: batched controlled-system dynamics (N = 2^20 states).

Strategy (v2):
  - Pure data parallel over 8 NeuronCores: contiguous slices of the batch axis.
  - dx1 = v1 and dx2 = v2 are exact passthroughs -> copied host-side (numpy),
    no device traffic at all.
  - dv1 / d_xc are linear in (x1,v1,x2,v2,xc,S=-sin(.5t)) -> TensorEngine:
    16 slices x 6 features = 96 contraction partitions, 2 outputs; matmul
    chunks of 512 cols go to 4 PSUM quadrants (tile_position col packing) so
    one [128,512] PSUM tile holds 4 chunks and one engine copy drains it.
  - dv2 = (K2/M2)*(Hn - phi_sel), Hn = F_net/K2 computed on DVE from bf16
    ribbons; phi = sgn(v2)*softplus(g0(v2)) (kinetic friction, b1=b2=0 so
    g0 is odd) decomposed as phi = sgn*(H(u) + v2*G(u)), u = v2^2, with
      H(u) = ln(2*cosh(g0(sqrt(u))/2))   (smooth in u)
      G(u) = g0(sqrt(u))/(2*sqrt(u))     (smooth in u)
    both fit as LOW-DEGREE polynomials in u with Gaussian-density weighting
    (v2 ~ N(0,1)); deg 1 suffices to land under the bf16 noise floor.
  - static branch (|v2| < 0.01): clip(-F_net/K2, +-L0) with L0 the stiction
    limit at v2=0 (linearization error ~1e-3), selected by predicated copy.
  - everything elementwise runs in bf16 ribbons [128,1024] (ribbon partition
    16m+s holds elements [s*8192 + m*1024, +1024)); outputs stored bf16 and
    upcast host-side (tolerance 2e-2, measured ~2e-3).
"""

import numpy as np

# physical system constants (match the reference)
M1, M2 = 1.0, 1.5
K1, K2 = 2.0, 3.0
C1, C2 = 0.5, 0.8
KARNOPP_DV = 0.01
REF_AMP, REF_OMEGA = 0.5, 0.5

N_CORES = 8
N_TOTAL = 1 << 20
N_CORE = N_TOTAL // N_CORES    # 131072
P = 128
F = N_CORE // P                # 1024
MB = P // 16                   # 8 ribbon column-blocks per slice

NSLICE = 16
SLICE_LEN = N_CORE // NSLICE   # 8192
NFEAT = 6                      # x1 v1 x2 v2 xc S
FP = NSLICE * NFEAT            # 96 feature partitions
NOUT = 2                       # dv1, d_xc
CHUNK = 512                    # matmul free-dim per chunk
NCHUNK = SLICE_LEN // CHUNK    # 16
QCH = 4                        # chunks per PSUM tile (quadrant packing)
NQ = NCHUNK // QCH             # 4 quarters

_compile_cache = {}


def _softplus(x):
    return np.log1p(np.exp(-np.abs(x))) + np.maximum(x, 0.0)


def _fit_friction(W1, b1, W2, b2, vmax):
    """Fit H(u), G(u) (see module docstring) as polynomials in u, Gaussian
    weighted. Returns coefficient lists (low order first) and L0."""
    W1 = W1.astype(np.float64).reshape(-1)
    b1 = b1.astype(np.float64).reshape(-1)
    W2 = W2.astype(np.float64)
    b2 = b2.astype(np.float64).reshape(-1)

    def gg(v, col):
        return np.tanh(np.outer(np.asarray(v, dtype=np.float64), W1) + b1) \
            @ W2[:, col] + b2[col]

    umax = vmax * vmax
    M = 4000
    wn = np.cos(np.pi * (np.arange(M) + 0.5) / M)
    uu = (wn + 1.0) / 2.0 * umax
    vv = np.sqrt(np.maximum(uu, 1e-14))
    g0v = gg(vv, 0)
    # odd/even decomposition targets (exact when b1 == 0; use the odd part of
    # g0 so nonzero-b1 weights still give the best-possible odd approximation)
    g0odd = (g0v - gg(-vv, 0)) / 2.0
    Ht = np.log(2.0 * np.cosh(g0odd / 2.0)) + (g0v + gg(-vv, 0)) / 2.0 * 0.0
    Gt = g0odd / (2.0 * vv)
    wgt = np.exp(-uu / 4.0) + 0.01

    def wfit(target, deg):
        V = np.vander(uu / umax, deg + 1, increasing=True)  # scaled for cond
        coef, *_ = np.linalg.lstsq(V * wgt[:, None], target * wgt, rcond=None)
        # unscale: coeff_k / umax^k
        return [coef[k] / (umax ** k) for k in range(deg + 1)]

    def werr(coefs, target):
        val = sum(c * uu ** k for k, c in enumerate(coefs))
        e = (val - target) * wgt
        return np.sqrt((e ** 2).mean())

    fits = {}
    for name, target in (("H", Ht), ("G", Gt)):
        for deg in (1, 2):
            cs = wfit(target, deg)
            if werr(cs, target) < 2.5e-2 or deg == 2:
                fits[name] = cs
                break

    L0 = float(_softplus(gg(np.array([0.0]), 1))[0])
    return dict(cH=fits["H"], cG=fits["G"], L0=L0)


def _make_wmat(consts):
    """lhsT [96, 32]: col (16*o + s) computes output o for slice s from that
    slice's 6 feature partitions (16f + s).

    Features: x1 v1 x2 v2 xc S, where S = sin(0.5 t - pi) = -sin(0.5 t).
    Outputs: 0: dv1, 1: d_xc.
    """
    K = consts["K"]; A = consts["A"]; p = consts["p"]
    B = np.zeros((NFEAT, NOUT), dtype=np.float64)
    # dv1 = (u - K1 x1 - C1 v1 - F_net)/M1, u = A xc + K e, e = -0.5 S - x2
    B[:, 0] = [-(K1 + K2) / M1, -(C1 + C2) / M1, (K2 - K) / M1,
               C2 / M1, A / M1, -0.5 * K / M1]
    # d_xc = e - p*xc = -0.5 S - x2 - p xc
    B[:, 1] = [0.0, 0.0, -1.0, 0.0, -p, -0.5]
    W = np.zeros((FP, NOUT * NSLICE), dtype=np.float32)
    for s in range(NSLICE):
        for o in range(NOUT):
            for f in range(NFEAT):
                W[NSLICE * f + s, NSLICE * o + s] = B[f, o]
    return W


def _build_program(consts):
    """Build the SPMD Bass program (same on all 8 cores)."""
    import concourse.bacc as bacc
    import concourse.mybir as mybir
    import bass_rust as _bass_rust
    from concourse import tile
    from concourse.hw_specs import get_activation_tables

    fp32 = mybir.dt.float32
    bf16 = mybir.dt.bfloat16
    Alu = mybir.AluOpType
    Act = mybir.ActivationFunctionType

    class _Bacc(bacc.Bacc):
        # All activations used (Sin, Sign, Square, Copy) live in
        # trig_and_small; strip them from every other set so exactly one
        # table load is emitted.
        def insert_act_table_loads(self):
            has_activation = any(
                isinstance(i, mybir.InstActivation)
                for b in self.main_func.blocks
                for i in b.instructions
            )
            if not has_activation:
                return
            tables = list(get_activation_tables(self.m.arch).items())
            fixed = []
            for name, funcs in tables:
                if name != "trig_and_small":
                    funcs = funcs - {Act.Square, Act.Sign, Act.Abs,
                                     Act.Identity, Act.Sin, Act.Copy}
                fixed.append((name, funcs))
            _bass_rust.insert_act_table_loads(self, fixed)

    cH = [float(np.float32(x)) for x in consts["cHn"]]
    cG = [float(np.float32(x)) for x in consts["cGn"]]
    dH = len(cH) - 1
    dG = len(cG) - 1
    L0K = float(np.float32(consts["L0"] / K2))
    thr = float(np.float32(KARNOPP_DV * KARNOPP_DV))

    nc = _Bacc()

    tb_d = nc.dram_tensor("tb", [N_CORE], bf16, kind="ExternalInput")
    zb_d = nc.dram_tensor("zb", [5, N_CORE], bf16, kind="ExternalInput")
    wm_d = nc.dram_tensor("wmat", [FP, NOUT * NSLICE], bf16, kind="ExternalInput")
    o1_d = nc.dram_tensor("o1", [N_CORE], bf16, kind="ExternalOutput")   # dv1
    o3_d = nc.dram_tensor("o3", [N_CORE], bf16, kind="ExternalOutput")   # dv2
    o4_d = nc.dram_tensor("o4", [N_CORE], bf16, kind="ExternalOutput")   # d_xc

    def rib(ap_row):
        return ap_row.rearrange("(p i) -> p i", p=P)

    o1_sl = o1_d[:].rearrange("(s q) -> s q", s=NSLICE)
    o4_sl = o4_d[:].rearrange("(s q) -> s q", s=NSLICE)

    with tile.TileContext(nc) as tc:
        with tc.tile_pool(name="sb", bufs=1) as pool, \
             tc.tile_pool(name="ps", bufs=1, space="PSUM") as psp:
            def tl(tag, dt=bf16, shape=(P, F)):
                return pool.tile(list(shape), dt, tag=tag, name=tag)

            T = tl("T")
            X1R = tl("X1R"); V1R = tl("V1R"); X2R = tl("X2R"); V2R = tl("V2R")
            FEAT = tl("FEAT", shape=(FP, SLICE_LEN))
            WM = tl("WM", shape=(FP, NOUT * NSLICE))

            # ---- loads: ribbons on the SWDGE queue (v2 first so the DVE
            # chain starts early), bulk FEAT + T + WM on the sync HWDGE ring
            nc.gpsimd.dma_start(out=V2R[:], in_=rib(zb_d[3, :]))
            nc.gpsimd.dma_start(out=X2R[:], in_=rib(zb_d[2, :]))
            nc.gpsimd.dma_start(out=X1R[:], in_=rib(zb_d[0, :]))
            nc.gpsimd.dma_start(out=V1R[:], in_=rib(zb_d[1, :]))
            nc.sync.dma_start(out=T[:], in_=rib(tb_d[:]))
            nc.sync.dma_start(out=WM[:], in_=wm_d[:])
            nc.sync.dma_start(
                out=FEAT[0:5 * NSLICE, :],
                in_=zb_d[0:5, :].rearrange("r (s q) -> (r s) q", s=NSLICE))

            # ---- Scalar: one act table; S = sin(t'), t' = 0.5t - pi ----
            S = tl("S")
            nc.scalar.activation(S[:], T[:], Act.Sin)
            Y = tl("Y")
            nc.scalar.activation(Y[:], V2R, Act.Square)
            SGN = tl("SGN")
            nc.scalar.activation(SGN[:], V2R, Act.Sign)

            # S -> FEAT slice rows (8 strided DMAs; the single-descriptor
            # rearrange form produced racy/garbage reads)
            for m in range(MB):
                nc.gpsimd.dma_start(
                    out=FEAT[5 * NSLICE:6 * NSLICE, m * F:(m + 1) * F],
                    in_=S[m::MB, :])

            # ---- DVE: friction + F_net chain (all bf16 ribbons) ----
            MASK = pool.tile([P, F], mybir.dt.uint8, tag="MASK", name="MASK")
            nc.vector.tensor_single_scalar(MASK[:], Y[:], thr, Alu.is_lt)

            # kinetic: P3 = sgn * ((cH1*y + cH0) + v2*(cG1*y + cG0))  [deg 1]
            # (general: A-part poly in y; t1 = G-poly(y)*v2)
            if dG == 1:
                QG = tl("QG")
                nc.vector.tensor_scalar(QG[:], Y[:], cG[1], cG[0], Alu.mult, Alu.add)
                T1 = tl("T1")
                nc.vector.tensor_tensor(T1[:], QG[:], V2R, Alu.mult)
            else:
                QG = tl("QG")
                nc.vector.tensor_scalar(QG[:], Y[:], cG[2], cG[1], Alu.mult, Alu.add)
                B2 = tl("B2")
                nc.vector.tensor_tensor(B2[:], QG[:], Y[:], Alu.mult)
                T1 = tl("T1")
                nc.vector.affine_mul_reduce(
                    T1[:], pool.tile([P, 1], fp32, tag="ACG", name="ACG")[:],
                    B2[:], V2R, 1.0, cG[0])
            if dH == 1:
                T2 = tl("T2")
                nc.vector.affine_then_add(T2[:], Y[:], T1[:], cH[1], cH[0])
            else:
                QH = tl("QH")
                nc.vector.tensor_scalar(QH[:], Y[:], cH[2], cH[1], Alu.mult, Alu.add)
                A2 = tl("A2")
                nc.vector.tensor_tensor(A2[:], QH[:], Y[:], Alu.mult)
                T2 = tl("T2")
                nc.vector.affine_then_add(T2[:], A2[:], T1[:], 1.0, cH[0])
            P3 = tl("P3")
            nc.vector.tensor_tensor(P3[:], T2[:], SGN[:], Alu.mult)

            # F_net: Hm = -F_net/K2 = (x2-x1) + (C2/K2)(v2-v1)
            FD1 = tl("FD1")
            nc.vector.tensor_tensor(FD1[:], X2R, X1R, Alu.subtract)
            FD2 = tl("FD2")
            nc.vector.tensor_tensor(FD2[:], V2R, V1R, Alu.subtract)
            FD2S = tl("FD2S")
            nc.vector.tensor_scalar(FD2S[:], FD2[:], C2 / K2, 0.0, Alu.mult, Alu.add)
            HM = tl("HM")
            nc.vector.tensor_tensor(HM[:], FD1[:], FD2S[:], Alu.add)

            # static: MM = clip(Hm, +-L0K) in ONE tensor_scalar (max, min)
            MM = tl("MM")
            nc.vector.tensor_scalar(MM[:], HM[:], -L0K, L0K, Alu.max, Alu.min)
            nc.vector.copy_predicated(P3[:], MASK[:], MM[:])

            # dv2 = (K2/M2) * (P3 - Hm)   [P3 = -phi/K2, Hm = -F_net/K2]
            D1 = tl("D1")
            nc.vector.tensor_tensor(D1[:], P3[:], HM[:], Alu.subtract)
            DV2 = tl("DV2")
            nc.vector.tensor_scalar(DV2[:], D1[:], K2 / M2, 0.0, Alu.mult, Alu.add)
            nc.sync.dma_start(out=rib(o3_d[:]), in_=DV2[:])

            # ---- TensorEngine: dv1 / d_xc (32-partition PSUM, ping-pong) ----
            QCOLS = QCH * CHUNK            # 2048 cols per PSUM tile
            PS = [psp.tile([NOUT * NSLICE, QCOLS], fp32, name=f"PS{i}", tag=f"PS{i}")
                  for i in range(2)]
            STG = tl("STG", shape=(NOUT * NSLICE, SLICE_LEN))
            for q in range(NQ):
                ps = PS[q % 2]
                for cc in range(QCH):
                    nc.tensor.matmul(ps[:, CHUNK * cc:CHUNK * (cc + 1)], WM[:],
                                     FEAT[:, QCOLS * q + CHUNK * cc:
                                          QCOLS * q + CHUNK * (cc + 1)],
                                     start=True, stop=True)
                # PSUM -> SBUF bf16, split between Scalar (front) and Vector
                sc = 1280
                nc.scalar.activation(STG[:, QCOLS * q:QCOLS * q + sc],
                                     ps[:, 0:sc], Act.Copy)
                nc.vector.tensor_copy(STG[:, QCOLS * q + sc:QCOLS * (q + 1)],
                                      ps[:, sc:QCOLS])
                # stores: slice-cols [2048q, 2048(q+1)) of each output row
                for o, osl in ((0, o1_sl), (1, o4_sl)):
                    nc.gpsimd.dma_start(
                        out=osl[:, QCOLS * q:QCOLS * (q + 1)],
                        in_=STG[NSLICE * o:NSLICE * (o + 1),
                                QCOLS * q:QCOLS * (q + 1)])

    nc.finalize()
    return nc


def _prepare(inputs):
    """Host-side constant folding + program build (cached on weight values)."""
    logK = np.float32(inputs["logK"]); logz = np.float32(inputs["logz"])
    logp = np.float32(inputs["logp"])
    W1 = np.asarray(inputs["W1"], dtype=np.float32)
    b1 = np.asarray(inputs["b1"], dtype=np.float32)
    W2 = np.asarray(inputs["W2"], dtype=np.float32)
    b2 = np.asarray(inputs["b2"], dtype=np.float32)
    v2 = np.asarray(inputs["z"][3], dtype=np.float32)
    vmax = float(np.abs(v2).max()) * 1.02 + 1e-3

    key = (logK.tobytes(), logz.tobytes(), logp.tobytes(), W1.tobytes(),
           b1.tobytes(), W2.tobytes(), b2.tobytes(), round(vmax, 2))
    if key in _compile_cache:
        return _compile_cache[key]

    K = np.float32(np.exp(logK))
    z_ctrl = np.float32(np.exp(logz))
    p_ctrl = np.float32(np.exp(logp))
    A = np.float32(K * (z_ctrl - p_ctrl))

    fit = _fit_friction(W1, b1, W2, b2, vmax)

    consts = dict(
        K=float(K), p=float(p_ctrl), A=float(A),
        cHn=[-c / K2 for c in fit["cH"]],     # chains evaluate -H/K2, -G/K2
        cGn=[-c / K2 for c in fit["cG"]],
        L0=fit["L0"],
    )
    wmat = _make_wmat(consts)
    nc = _build_program(consts)
    _compile_cache[key] = (nc, fit, wmat)
    return nc, fit, wmat


def _run(inputs, trace=False):
    from concourse.bass_utils import run_bass_kernel_spmd
    import ml_dtypes

    nc, _fit, wmat = _prepare(inputs)

    t = np.asarray(inputs["t"], dtype=np.float32)
    z = np.asarray(inputs["z"], dtype=np.float32)
    tb = (0.5 * t - np.float32(np.pi)).astype(ml_dtypes.bfloat16)
    zb = z.astype(ml_dtypes.bfloat16)
    wmat_b = np.ascontiguousarray(wmat.astype(ml_dtypes.bfloat16))
    in_maps = []
    for i in range(N_CORES):
        sl = slice(i * N_CORE, (i + 1) * N_CORE)
        in_maps.append({"tb": np.ascontiguousarray(tb[sl]),
                        "zb": np.ascontiguousarray(zb[:, sl]),
                        "wmat": wmat_b})

    res = run_bass_kernel_spmd(nc, in_maps, core_ids=list(range(N_CORES)),
                               trace=trace)
    out = np.empty((5, N_TOTAL), dtype=np.float32)
    out[0] = z[1]                      # dx1 = v1 (exact passthrough)
    out[2] = z[3]                      # dx2 = v2 (exact passthrough)
    for i in range(N_CORES):
        sl = slice(i * N_CORE, (i + 1) * N_CORE)
        r = res.results[i]
        out[1, sl] = r["o1"].astype(np.float32)
        out[3, sl] = r["o3"].astype(np.float32)
        out[4, sl] = r["o4"].astype(np.float32)
    return out, res


def kernel(**inputs):
    out, _res = _run(inputs, trace=False)
    return out

